# revision 2
# baseline (speedup 1.0000x reference)
"""CSAM (channel self-attention) Trainium2 kernel, v2.

Problem: x (16, 512, 64, 64) f32, gamma (1,) f32.
  q = x.reshape(B, C, N); energy = q @ q^T (per batch, C x C)
  attention = softmax(max(energy, -1, keepdims) - energy, -1) == softmax(-energy)
  out = gamma * (attention @ q) + x

Sharding: data-parallel over batch B=16 across 8 cores (2 batches/core).

Calibrated HW rates (measured via microbench, this tunnel): bf16 matmul
1 cyc/col @2.4GHz (78.6 TF/s), fp8e4 DoubleRow 2x that (157 TF/s), PE
transpose 1 cyc/col, DMA ~700 GB/s/core at 2MB descriptors. The v1
kernel was PE-bound (~123 us of PE columns); DMA is NOT the binding
resource at these rates.

v2 design:
  - bf16 I/O (host casts): 16.8 MB/core -> ~24 us DMA; plus a host-cast
    fp8e4 copy of x (+4.2 MB) so mm2 can run DoubleRow.
  - gram in fp8e4 DoubleRow: qT tiles cast bf16->fp8 for free during the
    PSUM->SBUF transpose evacuation (ACT engine); 16 DR matmuls per
    row-chunk contract 256 n-values each -> 2x PE throughput.
  - mm2 in fp8e4 DoubleRow: W~^T stored fp8 (free cast in its evac),
    paired d-chunks; rhs is the host-supplied fp8 natural-layout q.
  - gamma/S folded into W rows (per-partition scalar) before the W^T
    transpose, so no per-tile epilogue scaling.
  - residual: one DVE tensor_add per (q,m) over [128,1024] PSUM pairs:
    out_tile = o_psum(f32) + x(bf16) -> bf16, which also evacuates PSUM.
    With gamma=0, W~=0 so o=0 and the output is bit-exact bf16(x).
  - epilogue engine budget (per core): PE ~63 us (bound), DVE ~50,
    ACT ~23, Pool stores via SWDGE, SP load issue.
"""

import numpy as np

P = 128
C = 512
N = 4096
CO = C // P  # 4 c-chunks
NQ = 1024  # quarter-batch n extent
QN = N // NQ  # 4 quarters
KQ = NQ // P  # 8 n-chunks per quarter
NK = N // P  # 32 n-chunks per batch
NPAIR = NK // 2  # 16 DoubleRow pairs
BPC = 2  # batches per core
NCORES = 8

_CACHE = {}


def _build(repeats=1, fp8_gram=True, fp8_mm2=True, resid="dve", store_eng="gpsimd"):
    import concourse.bacc as bacc
    import concourse.tile as tile
    from concourse import mybir
    from concourse.masks import make_identity

    fp32 = mybir.dt.float32
    bf16 = mybir.dt.bfloat16
    fp8 = mybir.dt.float8e4
    Alu = mybir.AluOpType
    Act = mybir.ActivationFunctionType
    Ax = mybir.AxisListType
    DR = mybir.MatmulPerfMode.DoubleRow

    nc = bacc.Bacc("TRN2", debug=False, name="csam2")
    x = nc.dram_tensor("x", (BPC, C, N), bf16, kind="ExternalInput")
    gamma = nc.dram_tensor("gamma", (1,), fp32, kind="ExternalInput")
    out = nc.dram_tensor("out", (BPC, C, N), bf16, kind="ExternalOutput")
    if fp8_mm2:
        x8 = nc.dram_tensor("x8", (BPC, C, N), fp8, kind="ExternalInput")
        x8a = x8.ap()

    xa = x.ap()
    oa = out.ap()
    ga = gamma.ap()

    qt_dt = fp8 if fp8_gram else bf16

    with tile.TileContext(nc) as tc:
        with (
            tc.tile_pool(name="singles", bufs=1) as singles,
            tc.tile_pool(name="qb", bufs=2 * QN) as qb_pool,
            tc.tile_pool(name="q8", bufs=2) as q8_pool,
            tc.tile_pool(name="qt", bufs=2) as qt_pool,
            tc.tile_pool(name="wp", bufs=2) as w_pool,
            tc.tile_pool(name="wt", bufs=2) as wt_pool,
            tc.tile_pool(name="st", bufs=4) as st_pool,
            tc.tile_pool(name="ob", bufs=3) as ob_pool,
            tc.tile_pool(name="t_ps", bufs=2, space="PSUM") as t_psum,
            tc.tile_pool(name="e_ps", bufs=2, space="PSUM") as e_psum,
            tc.tile_pool(name="o_ps", bufs=2, space="PSUM") as o_psum,
        ):
            identity = singles.tile([P, P], bf16)
            make_identity(nc, identity)
            g_tile = singles.tile([P, 1], fp32)
            nc.gpsimd.dma_start(out=g_tile, in_=ga.to_broadcast((P, 1)))

            iters = [(r, b) for r in range(repeats) for b in range(BPC)]

            def emit_load(i):
                """Quarter loads + PE transposes; ACT evac-casts to qt."""
                _, b = iters[i]
                xv = xa[b].rearrange("(co ci) n -> ci co n", ci=P)
                qt_t = qt_pool.tile(
                    [P, NPAIR, 2, C], qt_dt, name=f"qt_{i}", tag="qt"
                )
                qbs = []
                if fp8_mm2:
                    x8v = x8a[b].rearrange("(co ci) n -> ci co n", ci=P)
                    q8_t = q8_pool.tile([P, CO, N], fp8, name=f"q8_{i}", tag="q8")
                for q in range(QN):
                    qb_t = qb_pool.tile(
                        [P, CO, NQ], bf16, name=f"qb_{i}_{q}", tag="qb"
                    )
                    nc.sync.dma_start(
                        out=qb_t, in_=xv[:, :, q * NQ : (q + 1) * NQ]
                    )
                    if fp8_mm2:
                        nc.sync.dma_start(
                            out=q8_t[:, :, q * NQ : (q + 1) * NQ],
                            in_=x8v[:, :, q * NQ : (q + 1) * NQ],
                        )
                    for k in range(KQ):
                        nk = q * KQ + k
                        tq = t_psum.tile(
                            [P, C], bf16, name=f"tq_{i}_{nk}", tag="tp"
                        )
                        for co in range(CO):
                            nc.tensor.transpose(
                                tq[:, co * P : (co + 1) * P],
                                qb_t[:, co, k * P : (k + 1) * P],
                                identity,
                            )
                        nc.scalar.copy(
                            out=qt_t[:, nk // 2, nk % 2, :], in_=tq[:]
                        )
                    qbs.append(qb_t)
                st = {"qbs": qbs, "qt": qt_t}
                if fp8_mm2:
                    st["q8"] = q8_t
                return st

            def emit_gram_softmax(i, st):
                """fp8-DR gram + per-row-chunk softmax + pipelined W~^T build."""
                qt_t = st["qt"]
                w_t = w_pool.tile([P, CO, C], bf16, name=f"w_{i}", tag="w")
                wst = w_pool.tile(
                    [P, CO, C], fp8 if fp8_mm2 else bf16,
                    name=f"wst_{i}", tag="wst",
                )
                ssum = st_pool.tile([P, CO], fp32, name=f"ssum_{i}", tag="ssum")
                gs = st_pool.tile([P, CO], fp32, name=f"gs_{i}", tag="gs")

                def emit_wst(m):
                    # wtil = (gamma/S[m-rows]) * W[m]; transpose into wst
                    wtil = wt_pool.tile(
                        [P, C], bf16, name=f"wtil_{i}_{m}", tag="wtil"
                    )
                    nc.vector.tensor_scalar_mul(
                        out=wtil, in0=w_t[:, m, :], scalar1=gs[:, m : m + 1]
                    )
                    tp = t_psum.tile([P, C], bf16, name=f"tp_{i}_{m}", tag="tp")
                    for dk in range(CO):
                        nc.tensor.transpose(
                            tp[:, dk * P : (dk + 1) * P],
                            wtil[:, dk * P : (dk + 1) * P],
                            identity,
                        )
                    nc.vector.tensor_copy(
                        out=wst[:, :, m * P : (m + 1) * P],
                        in_=tp.rearrange("p (dk c) -> p dk c", dk=CO),
                    )

                for m in range(CO):
                    e_t = e_psum.tile([P, C], fp32, name=f"e_{i}_{m}", tag="e")
                    if fp8_gram:
                        for j in range(NPAIR):
                            nc.tensor.matmul(
                                e_t,
                                lhsT=qt_t[:, j, :, m * P : (m + 1) * P],
                                rhs=qt_t[:, j, :, :],
                                start=(j == 0),
                                stop=(j == NPAIR - 1),
                                perf_mode=DR,
                            )
                    else:
                        for j in range(NPAIR):
                            for j2 in range(2):
                                nc.tensor.matmul(
                                    e_t,
                                    lhsT=qt_t[:, j, j2, m * P : (m + 1) * P],
                                    rhs=qt_t[:, j, j2, :],
                                    start=(j == 0 and j2 == 0),
                                    stop=(j == NPAIR - 1 and j2 == 1),
                                )
                    mmin = st_pool.tile(
                        [P, 1], fp32, name=f"mmin_{i}_{m}", tag="mmin"
                    )
                    nc.vector.tensor_reduce(
                        out=mmin, in_=e_t, axis=Ax.X, op=Alu.min
                    )
                    nc.scalar.activation(
                        out=w_t[:, m, :],
                        in_=e_t,
                        func=Act.Exp,
                        bias=mmin,
                        scale=-1.0,
                        accum_out=ssum[:, m : m + 1],
                    )
                    # gs = gamma / S for this row-chunk
                    nc.vector.reciprocal(
                        out=gs[:, m : m + 1], in_=ssum[:, m : m + 1]
                    )
                    nc.vector.tensor_scalar_mul(
                        out=gs[:, m : m + 1],
                        in0=gs[:, m : m + 1],
                        scalar1=g_tile,
                    )
                    if m > 0:
                        emit_wst(m - 1)
                emit_wst(CO - 1)
                st["wst"] = wst

            def emit_tail(i, st):
                """mm2 (fp8 DR) + DVE residual add (also the PSUM evac)."""
                _, b = iters[i]
                ov = oa[b].rearrange("(co ci) n -> ci co n", ci=P)
                wst = st["wst"]
                qbs = st["qbs"]
                q8_t = st.get("q8")
                nsl = 512
                nslc = NQ // nsl  # 2
                store_engine = dict(
                    gpsimd=nc.gpsimd, sync=nc.sync, scalar=nc.scalar
                )[store_eng]
                if fp8_mm2:
                    wstv = wst.rearrange("p (dj j2) c -> p dj j2 c", j2=2)
                    q8v = q8_t.rearrange("p (dj j2) n -> p dj j2 n", j2=2)
                for q in range(QN):
                    ob_t = ob_pool.tile(
                        [P, CO, NQ], bf16, name=f"ob_{i}_{q}", tag="ob"
                    )
                    for m in range(CO):
                        o_ps = o_psum.tile(
                            [P, NQ], fp32, name=f"o_{i}_{q}_{m}", tag="o"
                        )
                        for sl in range(nslc):
                            o_half = o_ps[:, sl * nsl : (sl + 1) * nsl]
                            if fp8_mm2:
                                for dj in range(CO // 2):
                                    nc.tensor.matmul(
                                        o_half,
                                        lhsT=wstv[:, dj, :, m * P : (m + 1) * P],
                                        rhs=q8v[
                                            :,
                                            dj,
                                            :,
                                            q * NQ + sl * nsl : q * NQ
                                            + (sl + 1) * nsl,
                                        ],
                                        start=(dj == 0),
                                        stop=(dj == CO // 2 - 1)
                                        and resid == "dve",
                                        perf_mode=DR,
                                    )
                            else:
                                for dk in range(CO):
                                    nc.tensor.matmul(
                                        o_half,
                                        lhsT=wst[:, dk, m * P : (m + 1) * P],
                                        rhs=qbs[q][
                                            :, dk, sl * nsl : (sl + 1) * nsl
                                        ],
                                        start=(dk == 0),
                                        stop=(dk == CO - 1) and resid == "dve",
                                    )
                            if resid == "pe":
                                nc.tensor.matmul(
                                    o_half,
                                    lhsT=identity,
                                    rhs=qbs[q][:, m, sl * nsl : (sl + 1) * nsl],
                                    start=False,
                                    stop=True,
                                )
                        if resid == "dve":
                            nc.vector.tensor_add(
                                out=ob_t[:, m, :],
                                in0=o_ps,
                                in1=qbs[q][:, m, :],
                            )
                        else:
                            eng = nc.vector if m % 2 else nc.scalar
                            eng_copy = (
                                eng.tensor_copy if eng is nc.vector else eng.copy
                            )
                            eng_copy(out=ob_t[:, m, :], in_=o_ps)
                    store_engine.dma_start(
                        out=ov[:, :, q * NQ : (q + 1) * NQ], in_=ob_t
                    )

            states = {0: emit_load(0)}
            for i in range(len(iters)):
                emit_gram_softmax(i, states[i])
                if i + 1 < len(iters):
                    states[i + 1] = emit_load(i + 1)
                emit_tail(i, states[i])
                del states[i]

    nc.compile()
    return nc


def _get(repeats=1, **kw):
    key = (repeats, tuple(sorted(kw.items())))
    if key not in _CACHE:
        _CACHE[key] = _build(repeats, **kw)
    return _CACHE[key]


_FP8_MM2 = True


def _run(nc, x, gamma):
    import ml_dtypes
    from concourse.bass_utils import run_bass_kernel_spmd

    xr = np.asarray(x).reshape(-1, C, N)
    xb = xr.astype(ml_dtypes.bfloat16)
    g = np.ascontiguousarray(np.asarray(gamma, dtype=np.float32)).reshape(1)
    x8 = xb.astype(ml_dtypes.float8_e4m3fn) if _FP8_MM2 else None
    in_maps = []
    for c in range(NCORES):
        m = {"x": xb[BPC * c : BPC * (c + 1)], "gamma": g}
        if _FP8_MM2:
            m["x8"] = x8[BPC * c : BPC * (c + 1)]
        in_maps.append(m)
    res = run_bass_kernel_spmd(nc, in_maps, core_ids=list(range(NCORES)))
    outs = np.concatenate([r["out"] for r in res.results], axis=0)
    return outs.astype(np.float32)


def kernel(x, gamma):
    import time

    x = np.asarray(x)
    B, Cc, H, W = x.shape
    nc = _get(1)
    last = None
    for attempt in range(3):
        try:
            out = _run(nc, x, gamma)
            return out.reshape(B, Cc, H, W)
        except Exception as e:  # noqa: BLE001
            last = e
            time.sleep(2.0)
            try:
                import jax

                jax.clear_caches()
                jax.extend.backend.clear_backends()
            except Exception:  # noqa: BLE001
                pass
    raise last


if __name__ == "__main__":
    rng = np.random.default_rng(0)
    x = rng.standard_normal((16, 512, 64, 64), dtype=np.float32)
    gamma = np.array([0.7], dtype=np.float32)
    o = kernel(x, gamma)
    print(o.shape, o.dtype)
